# revision 32
# baseline (speedup 1.0000x reference)
"""Bass/Trainium2 kernel for nn_DiagWinAttention (swin-style windowed attention).

v4 design (all-bf16 operands, f32 PSUM; every engine pipelined, no GPSIMD):
  scores: ONE PSUM bank per pair accumulates bias (identity-matmul adder,
          start=True) + mask (identity adder, rhs broadcast over heads) +
          12 QK^T matmuls -> S^T[j,(h,i)].  Partial-array QK weights use
          only base partitions {0,64}: heads 0-3 run as K=64 against
          per-head zero-padded q blocks, heads 4-5 as K=32 at base 64.
          (HW bug found by bisection: a partial-array LDWEIGHTS at base 32
          sharing a PSUM bank with any other-base matmul crashes the
          device; {0,64} mixing is safe - see repro history.)
  ACT:    one Exp drains the bank -> P~ = exp(S+b+m) bf16 (unnormalized
          softmax; denominators recovered via ones-column in V)
  PE:     12 AV matmuls (ones-augmented V) -> attn out + denominators,
          4 pairs sharing one PSUM bank
  DVE:    batched per 4 pairs: reciprocal, x = av*rec + q*scale,
          bn_stats/bn_aggr, then rstd = rsqrt(var+eps) via bit-trick seed
          (int32 shift/xor/add ALU ops) + 1 Newton step -- avoids ACT Sqrt
          entirely so ACT stays on the exp_and_others table set (a single
          exp<->sqrt alternation costs ~5.3us in table reloads)
  PE:     per-pair transpose into a shared PSUM tile; proj matmul batched
          over 4 pairs (N=512); ACT Identity-copy (+LN beta/proj bias
          folded into a per-partition bias) -> yT bf16

Measured: 376 us/rep/core on HW (8 cores, 512 windows each) vs 274 ms for
the v2 baseline; rel err 6.1e-3 vs f32 reference (limit 2e-2), dominated
by bf16 input rounding.

Sharding: pure data-parallel over nw across 8 cores (512 windows/core).
"""

import numpy as np
import ml_dtypes
from contextlib import ExitStack

import concourse.bacc as bacc
import concourse.tile as tile
from concourse import mybir
from concourse.bass import ts as _ts
from concourse.bass_utils import run_bass_kernel_spmd

N_CORES = 8
NW = 4096
N = 64          # tokens per window
E = 96          # embed
NH = 6          # heads
CH = 16         # head dim
SCALE = CH ** -0.5
EPS = 1e-5
F32 = mybir.dt.float32
BF16 = mybir.dt.bfloat16
BF = ml_dtypes.bfloat16

PAIR_T = 128          # tokens per pair tile (2 windows)
CP = 16               # pairs per chunk
PB = 262              # per-pair block cols: vp(102) + em(64) + qs(96)
PROJ_G = 4            # pairs per batched proj matmul


def _rel_position_index():
    ws = (8, 8)
    coords = np.stack(np.meshgrid(np.arange(ws[0]), np.arange(ws[1]), indexing="ij"))
    cf = coords.reshape(2, -1)
    rel = cf[:, :, None] - cf[:, None, :]
    rel = np.moveaxis(rel, 0, -1).astype(np.int64)
    rel[..., 0] += ws[0] - 1
    rel[..., 0] *= 2 * ws[1] - 1
    rel[..., 1] += ws[1] - 1
    return rel.sum(-1).reshape(-1)


def build_nc(nw_core: int, reps: int = 1, cp: int = CP, dyn: bool = True, stage: int = 3):
    tok = nw_core * N
    pairs = tok // PAIR_T
    cp = min(cp, pairs)
    n_chunk = pairs // cp
    assert pairs % cp == 0
    T = cp * PAIR_T                      # tokens per chunk
    X = 5 * T + PB * cp                  # slab cols (bf16)
    proj_g = min(PROJ_G, cp)
    assert cp % proj_g == 0
    GRP = min(4, n_chunk)
    assert n_chunk % GRP == 0
    n_iter = n_chunk // GRP

    nc = bacc.Bacc("TRN2", target_bir_lowering=False, debug=False)

    slab_d = nc.dram_tensor("slab", [(n_iter + 1) * 128, GRP * X], BF16,
                        kind="ExternalInput")
    biasT_d = nc.dram_tensor("biasT", [128, NH * N], BF16, kind="ExternalInput")
    wt_d = nc.dram_tensor("wt", [E, E], BF16, kind="ExternalInput")
    identb_d = nc.dram_tensor("identb", [128, 128], BF16, kind="ExternalInput")
    coff_d = nc.dram_tensor("coff", [E, 1], F32, kind="ExternalInput")
    yT_d = nc.dram_tensor("yT", [E, tok], BF16, kind="ExternalOutput")
    dbg_d = nc.dram_tensor("dbg", [128, 512], F32, kind="ExternalOutput") \
        if stage < 3 else None

    Exp = mybir.ActivationFunctionType.Exp
    Ln = mybir.ActivationFunctionType.Ln
    Ident = mybir.ActivationFunctionType.Identity

    with tile.TileContext(nc) as tc, ExitStack() as ctx:
        consts = ctx.enter_context(tc.tile_pool(name="consts", bufs=1))
        slabp = ctx.enter_context(tc.tile_pool(name="slabp", bufs=1))
        ytp = ctx.enter_context(tc.tile_pool(name="ytp", bufs=2))
        work = ctx.enter_context(tc.tile_pool(name="work", bufs=6))
        ps_s = ctx.enter_context(tc.tile_pool(name="ps_s", bufs=2, space="PSUM"))
        ps_a = ctx.enter_context(tc.tile_pool(name="ps_a", bufs=2, space="PSUM"))
        ps_t = ctx.enter_context(tc.tile_pool(name="ps_t", bufs=2, space="PSUM"))
        ps_z = ctx.enter_context(tc.tile_pool(name="ps_z", bufs=2, space="PSUM"))

        biasT = consts.tile([128, NH * N], BF16, tag="biasT")
        nc.sync.dma_start(out=biasT, in_=biasT_d[:, :])
        wt = consts.tile([E, E], BF16, tag="wt")
        nc.sync.dma_start(out=wt, in_=wt_d[:, :])
        identb = consts.tile([128, 128], BF16, tag="identb")
        nc.sync.dma_start(out=identb, in_=identb_d[:, :])
        coff = consts.tile([E, 1], F32, tag="coff")
        nc.sync.dma_start(out=coff, in_=coff_d[:, :])
        eps_t = consts.tile([128, 1], F32, tag="eps")
        nc.vector.memset(eps_t, EPS)

        def chunk_body(slab, yt, yoff, uniq):
            kT = slab[0:E, 0:T]
            qb = [slab[0:E, (1 + j) * T:(2 + j) * T] for j in range(4)]

            for g in range(cp // proj_g):
                xnT = work.tile([E, proj_g * PAIR_T], BF16, tag="xnT",
                                name=f"xnT_{uniq}_{g}")
                mvg = work.tile([PAIR_T, proj_g, 2], F32, tag="mvg",
                                name=f"mvg_{uniq}_{g}")
                bg = 5 * T + PB * (g * proj_g)
                blk4 = slab[:, bg:bg + proj_g * PB].rearrange(
                    "p (b r) -> p b r", b=proj_g)
                av4 = ps_a.tile([PAIR_T, proj_g, NH * 17], F32, tag="av",
                                name=f"av_{uniq}_{g}")
                for pg in range(proj_g):
                    p = g * proj_g + pg
                    t0 = PAIR_T * p
                    b0 = 5 * T + PB * p
                    vp = slab[:, b0:b0 + 102]
                    em = slab[:, b0 + 102:b0 + 166]

                    # ---- scores: one PSUM bank; partial-array QK weights use
                    # only bases {0, 64} (heads 0-3 as K=64 with per-head
                    # zero-padded q, heads 4-5 as K=32 at base 64) — mixing a
                    # base-32 load with others in one bank crashes the HW
                    sT = ps_s.tile([PAIR_T, NH * N], F32, tag="sT",
                                   name=f"sT_{uniq}_{p}")
                    nc.tensor.matmul(out=sT[:, :], lhsT=identb[:, :],
                                     rhs=biasT[:, :],
                                     start=True, stop=False,
                                     skip_group_check=True)
                    em_b = em.unsqueeze(1).broadcast_to([PAIR_T, NH, N])
                    nc.tensor.matmul(out=sT[:].rearrange("p (h i) -> p h i", h=NH),
                                     lhsT=identb[:, :], rhs=em_b,
                                     start=False, stop=False,
                                     skip_group_check=True)
                    for s in range(2):
                        ts0 = t0 + 64 * s
                        for h in range(NH):
                            last = (s == 1 and h == NH - 1)
                            if h < 4:
                                lhs = kT[0:64, ts0:ts0 + 64]
                                rhs = qb[h][0:64, ts0:ts0 + 64]
                            else:
                                lhs = kT[64:96, ts0:ts0 + 64]
                                rhs = qb[h - 4][64:96, ts0:ts0 + 64]
                            nc.tensor.matmul(
                                out=sT[64 * s:64 * s + 64, N * h:N * h + N],
                                lhsT=lhs, rhs=rhs,
                                start=False, stop=last,
                                skip_group_check=True)

                    # ---- P~ = exp(S+b+m), one drain -> bf16
                    e_t = work.tile([PAIR_T, NH * N], BF16, tag="e",
                                    name=f"e_{uniq}_{p}")
                    nc.scalar.activation(out=e_t[:, :], in_=sT[:, :], func=Exp)

                    if stage == 1:
                        dbge = work.tile([128, NH * N], F32, tag="dbge",
                                         name=f"dbge_{uniq}_{p}")
                        nc.vector.tensor_copy(out=dbge[:, :], in_=e_t[:, :])
                        nc.sync.dma_start(out=dbg_d[:, 0:NH * N], in_=dbge)
                        continue

                    # ---- AV with ones-augmented V: out + denominators
                    for s in range(2):
                        for h in range(NH):
                            nc.tensor.matmul(
                                out=av4[64 * s:64 * s + 64, pg, 17 * h:17 * h + 17],
                                lhsT=e_t[64 * s:64 * s + 64, N * h:N * h + N],
                                rhs=vp[64 * s:64 * s + 64, 17 * h:17 * h + 17])

                if stage == 1:
                    continue
                # ---- batched post-AV for the whole group (4 pairs at once)
                av_v = av4[:].rearrange("p g (h c) -> p g h c", h=NH)
                rec = work.tile([PAIR_T, proj_g, NH], F32, tag="rec",
                                name=f"rec_{uniq}_{g}")
                nc.vector.reciprocal(out=rec[:, :, :], in_=av_v[:, :, :, 16])
                x4 = work.tile([PAIR_T, proj_g, E], F32, tag="x",
                               name=f"x_{uniq}_{g}")
                x_v = x4[:].rearrange("p g (h c) -> p g h c", h=NH)
                rec_b = rec[:].unsqueeze(3).broadcast_to([PAIR_T, proj_g, NH, CH])
                nc.vector.tensor_tensor(out=x_v, in0=av_v[:, :, :, 0:16], in1=rec_b,
                                        op=mybir.AluOpType.mult)
                qs4 = blk4[:, :, 166:262]
                nc.vector.tensor_tensor(out=x4[:, :, :], in0=x4[:, :, :], in1=qs4,
                                        op=mybir.AluOpType.add)

                if stage == 2:
                    nc.sync.dma_start(out=dbg_d[:, 0:proj_g * E],
                                      in_=x4.reshape_hint if False else x4[:, :, :])
                    continue

                # ---- LayerNorm stats (walrus: bn_stats out must be 6/partition)
                for pg in range(proj_g):
                    stats = work.tile([PAIR_T, 6], F32, tag="stats",
                                      name=f"st_{uniq}_{g}_{pg}")
                    nc.vector.bn_stats(out=stats[:, :], in_=x4[:, pg, :])
                    nc.vector.bn_aggr(out=mvg[:, pg, :], in_=stats[:, :])

                # ---- rstd = rsqrt(var+eps): bit-trick seed + 1 Newton step,
                #      all int/mult DVE ALU ops (keeps ACT on one table set)
                tg = work.tile([PAIR_T, proj_g], F32, tag="tg",
                               name=f"tg_{uniq}_{g}")
                nc.vector.tensor_scalar(out=tg[:, :], in0=mvg[:, :, 1],
                                        scalar1=EPS, scalar2=None,
                                        op0=mybir.AluOpType.add)
                y0 = work.tile([PAIR_T, proj_g], F32, tag="y0",
                               name=f"y0_{uniq}_{g}")
                nc.vector.tensor_scalar(out=y0[:].bitcast(mybir.dt.int32),
                                        in0=tg[:].bitcast(mybir.dt.int32),
                                        scalar1=1, scalar2=-1,
                                        op0=mybir.AluOpType.logical_shift_right,
                                        op1=mybir.AluOpType.bitwise_xor)
                nc.vector.tensor_scalar(out=y0[:].bitcast(mybir.dt.int32),
                                        in0=y0[:].bitcast(mybir.dt.int32),
                                        scalar1=0x5F3759E0, scalar2=None,
                                        op0=mybir.AluOpType.add)
                d_t = work.tile([PAIR_T, proj_g], F32, tag="dt",
                                name=f"dt_{uniq}_{g}")
                nc.vector.tensor_tensor(out=d_t[:, :], in0=y0[:, :], in1=y0[:, :],
                                        op=mybir.AluOpType.mult)
                nc.vector.tensor_tensor(out=d_t[:, :], in0=d_t[:, :], in1=tg[:, :],
                                        op=mybir.AluOpType.mult)
                nc.vector.tensor_scalar(out=d_t[:, :], in0=d_t[:, :],
                                        scalar1=-0.5, scalar2=1.5,
                                        op0=mybir.AluOpType.mult,
                                        op1=mybir.AluOpType.add)
                rstd = work.tile([PAIR_T, proj_g], F32, tag="rstd",
                                 name=f"rstd_{uniq}_{g}")
                nc.vector.tensor_tensor(out=rstd[:, :], in0=y0[:, :], in1=d_t[:, :],
                                        op=mybir.AluOpType.mult)

                xnT_p = ps_t.tile([E, proj_g * PAIR_T], BF16, tag="tr",
                                  name=f"tr_{uniq}_{g}")
                for pg in range(proj_g):
                    p = g * proj_g + pg
                    xn = work.tile([PAIR_T, E], BF16, tag="xn", name=f"xn_{uniq}_{p}")
                    nc.vector.tensor_scalar(out=xn[:, :], in0=x4[:, pg, :],
                                            scalar1=mvg[:, pg, 0:1],
                                            scalar2=rstd[:, pg:pg + 1],
                                            op0=mybir.AluOpType.subtract,
                                            op1=mybir.AluOpType.mult)
                    # ---- transpose for proj (4 pairs share one PSUM tile)
                    nc.tensor.transpose(
                        out=xnT_p[:, PAIR_T * pg:PAIR_T * (pg + 1)], in_=xn[:, :],
                        identity=identb[:, :])
                nc.vector.tensor_copy(out=xnT[:, :], in_=xnT_p[:, :])

                # ---- proj over 4 pairs at once (N=512)
                zT = ps_z.tile([E, proj_g * PAIR_T], F32, tag="zT",
                               name=f"zT_{uniq}_{g}")
                nc.tensor.matmul(out=zT[:, :], lhsT=wt[:, :], rhs=xnT[:, :])
                c0 = yoff + proj_g * PAIR_T * g
                nc.scalar.activation(out=yt[:, c0:c0 + proj_g * PAIR_T],
                                     in_=zT[:, :], func=Ident, bias=coff[:, :])

        def group_body(ci, uniq, slab0):
            # GRP chunks per loop iteration.  Chunk 0's slab was prefetched
            # by the previous iteration (prologue for iter 0); chunks 1..GRP-1
            # DMA here and overlap chunk 0's compute.  At body end we prefetch
            # the NEXT iteration's chunk 0 into the same tile (slab_d has one
            # zero-padded tail block so ci+1 never reads out of bounds).
            slabs = [slab0]
            for j in range(1, GRP):
                sl = slabp.tile([128, X], BF16, tag=f"slab{j}",
                                name=f"slab_{uniq}_{j}")
                if dyn:
                    nc.sync.dma_start(out=sl,
                                      in_=slab_d[_ts(ci, 128), j * X:(j + 1) * X])
                else:
                    nc.sync.dma_start(
                        out=sl, in_=slab_d[ci * 128:(ci + 1) * 128,
                                           j * X:(j + 1) * X])
                slabs.append(sl)
            yt = ytp.tile([E, GRP * T], BF16, tag="yt", name=f"yt_{uniq}") \
                if stage >= 3 else None
            for j in range(GRP):
                chunk_body(slabs[j], yt, j * T, f"{uniq}_{j}")
            if dyn:
                nc.sync.dma_start(out=slab0,
                                  in_=slab_d[_ts(ci + 1, 128), 0:X])
            else:
                nc.sync.dma_start(
                    out=slab0, in_=slab_d[(ci + 1) * 128:(ci + 2) * 128, 0:X])
            if stage >= 3:
                if dyn:
                    nc.sync.dma_start(out=yT_d[:, _ts(ci, GRP * T)], in_=yt)
                else:
                    nc.sync.dma_start(
                        out=yT_d[:, ci * GRP * T:(ci + 1) * GRP * T], in_=yt)

        if dyn:
            hints = tuple(mybir.ALL_ENGINES)
            with tc.For_i(0, reps) as _rep:
                slab0 = slabp.tile([128, X], BF16, tag="slab0", name="slab0_d")
                nc.sync.dma_start(out=slab0, in_=slab_d[0:128, 0:X])
                with tc.For_i(0, n_iter, hint_engines=hints) as ci:
                    group_body(ci, "d", slab0)
        else:
            for rep in range(reps):
                slab0 = slabp.tile([128, X], BF16, tag="slab0",
                                   name=f"slab0_{rep}")
                nc.sync.dma_start(out=slab0, in_=slab_d[0:128, 0:X])
                for ci in range(n_iter):
                    group_body(ci, f"{rep}_{ci}", slab0)

    nc.compile()
    return nc


def prepare_inputs(query, key, value, mask, bias_table, norm_gamma, norm_beta,
                   proj_w, proj_b, cp: int = CP):
    """Host-side data prep. Returns arrays shardable per-core along chunk axis."""
    nw = query.shape[0]
    tok = nw * N
    q2 = (query.astype(np.float32) * SCALE).reshape(tok, E)
    qT = q2.T                                          # [E, tok] f32 view
    kT = np.ascontiguousarray(
        key.astype(np.float32).reshape(tok, E).T).astype(BF)
    qb = np.zeros((4, E, tok), BF)
    for h in range(4):
        qb[h, CH * h:CH * h + CH] = qT[CH * h:CH * h + CH].astype(BF)
    qb[0, 64:80] = qT[64:80].astype(BF)     # head 4 rides block 0 rows 64-79
    qb[1, 80:96] = qT[80:96].astype(BF)     # head 5 rides block 1 rows 80-95

    vp = np.empty((tok, NH * 17), BF)
    v2 = value.reshape(tok, E)
    for h in range(NH):
        vp[:, 17 * h:17 * h + 16] = v2[:, 16 * h:16 * h + 16].astype(BF)
        vp[:, 17 * h + 16] = 1.0

    em = np.ascontiguousarray(
        mask.astype(np.float32).transpose(0, 2, 1)).reshape(tok, N).astype(BF)
    qs = q2.astype(BF)

    pairs = tok // PAIR_T
    cp = min(cp, pairs)
    n_chunk_total = pairs // cp
    T = cp * PAIR_T
    X = 5 * T + PB * cp

    slab = np.zeros((n_chunk_total, 128, X), BF)
    for ci in range(n_chunk_total):
        a = ci * T
        slab[ci, 0:E, 0:T] = kT[:, a:a + T]
        for j in range(4):
            slab[ci, 0:E, (1 + j) * T:(2 + j) * T] = qb[j, :, a:a + T]
        for p in range(cp):
            b0 = 5 * T + PB * p
            r = a + p * PAIR_T
            slab[ci, :, b0:b0 + 102] = vp[r:r + PAIR_T]
            slab[ci, :, b0 + 102:b0 + 166] = em[r:r + PAIR_T]
            slab[ci, :, b0 + 166:b0 + 262] = qs[r:r + PAIR_T]

    rel = _rel_position_index()
    bias = bias_table[rel].reshape(N, N, NH)                 # [i, j, h]
    bjhi = np.ascontiguousarray(bias.transpose(1, 2, 0)).reshape(N, NH * N)
    biasT = np.vstack([bjhi, bjhi]).astype(BF)               # [128, 384]

    weff = (proj_w * norm_gamma[None, :]).astype(np.float32)
    wt = np.ascontiguousarray(weff.T).astype(BF)             # [c, o]
    coff = (norm_beta @ proj_w.T + proj_b).astype(np.float32).reshape(E, 1)

    return {
        "slab": slab, "biasT": biasT, "wt": wt, "coff": coff,
        "identb": np.eye(128, dtype=BF),
    }


def core_in_maps(full, n_cores=N_CORES):
    n_chunk_total = full["slab"].shape[0]
    chunks_c = n_chunk_total // n_cores
    grp = min(4, chunks_c)
    maps = []
    for c in range(n_cores):
        sl = full["slab"][c * chunks_c:(c + 1) * chunks_c]
        X = sl.shape[2]
        sl = np.ascontiguousarray(
            sl.reshape(chunks_c // grp, grp, 128, X).transpose(0, 2, 1, 3))
        sl = sl.reshape(chunks_c // grp * 128, grp * X)
        sl = np.concatenate([sl, np.zeros((128, grp * X), BF)], axis=0)
        maps.append({
            "slab": sl,
            "biasT": full["biasT"], "wt": full["wt"],
            "coff": full["coff"], "identb": full["identb"],
        })
    return maps


_NC_CACHE = {}


def kernel(**inputs) -> np.ndarray:
    nw = inputs["query"].shape[0]
    assert nw % N_CORES == 0
    nw_c = nw // N_CORES

    full = prepare_inputs(**inputs)
    in_maps = core_in_maps(full)

    if nw_c not in _NC_CACHE:
        _NC_CACHE[nw_c] = build_nc(nw_c)
    nc = _NC_CACHE[nw_c]

    res = run_bass_kernel_spmd(nc, in_maps, core_ids=list(range(N_CORES)))
    yT = np.concatenate(
        [res.results[c]["yT"].astype(np.float32) for c in range(N_CORES)], axis=1)
    return np.ascontiguousarray(yT.T).reshape(nw, 8, 8, E).astype(np.float32)


if __name__ == "__main__":
    rng = np.random.default_rng(0)
    inputs = {
        "query": rng.standard_normal((NW, N, E), dtype=np.float32),
        "key": rng.standard_normal((NW, N, E), dtype=np.float32),
        "value": rng.standard_normal((NW, N, E), dtype=np.float32),
        "mask": rng.standard_normal((NW, N, N), dtype=np.float32),
        "bias_table": (rng.standard_normal((225, NH)) * 0.02).astype(np.float32),
        "norm_gamma": np.ones(E, np.float32),
        "norm_beta": np.zeros(E, np.float32),
        "proj_w": (rng.standard_normal((E, E)) * 0.02).astype(np.float32),
        "proj_b": np.zeros(E, np.float32),
    }
    print(kernel(**inputs).shape)


# revision 33
# speedup vs baseline: 1.1636x; 1.1636x over previous
"""Bass/Trainium2 kernel for nn_DiagWinAttention (swin-style windowed attention).

v4 design (all-bf16 operands, f32 PSUM; every engine pipelined, no GPSIMD):
  scores: ONE PSUM bank per pair accumulates bias (identity-matmul adder,
          start=True) + mask (identity adder, rhs broadcast over heads) +
          12 QK^T matmuls -> S^T[j,(h,i)].  Partial-array QK weights use
          only base partitions {0,64}: heads 0-3 run as K=64 against
          per-head zero-padded q blocks, heads 4-5 as K=32 at base 64.
          (HW bug found by bisection: a partial-array LDWEIGHTS at base 32
          sharing a PSUM bank with any other-base matmul crashes the
          device; {0,64} mixing is safe - see repro history.)
  ACT:    one Exp drains the bank -> P~ = exp(S+b+m) bf16 (unnormalized
          softmax; denominators recovered via ones-column in V)
  PE:     12 AV matmuls (ones-augmented V) -> attn out + denominators,
          4 pairs sharing one PSUM bank
  DVE:    batched per 4 pairs: reciprocal, x = av*rec + q*scale,
          bn_stats/bn_aggr, then rstd = rsqrt(var+eps) via bit-trick seed
          (int32 shift/xor/add ALU ops) + 1 Newton step -- avoids ACT Sqrt
          entirely so ACT stays on the exp_and_others table set (a single
          exp<->sqrt alternation costs ~5.3us in table reloads)
  PE:     per-pair transpose into a shared PSUM tile; proj matmul batched
          over 4 pairs (N=512); ACT Identity-copy (+LN beta/proj bias
          folded into a per-partition bias) -> yT bf16

Measured: 376 us/rep/core on HW (8 cores, 512 windows each) vs 274 ms for
the v2 baseline; rel err 6.1e-3 vs f32 reference (limit 2e-2), dominated
by bf16 input rounding.

Sharding: pure data-parallel over nw across 8 cores (512 windows/core).
"""

import numpy as np
import ml_dtypes
from contextlib import ExitStack

import concourse.bacc as bacc
import concourse.tile as tile
from concourse import mybir
from concourse.bass import ts as _ts
from concourse.bass_utils import run_bass_kernel_spmd

N_CORES = 8
NW = 4096
N = 64          # tokens per window
E = 96          # embed
NH = 6          # heads
CH = 16         # head dim
SCALE = CH ** -0.5
EPS = 1e-5
F32 = mybir.dt.float32
BF16 = mybir.dt.bfloat16
BF = ml_dtypes.bfloat16

PAIR_T = 128          # tokens per pair tile (2 windows)
CP = 16               # pairs per chunk
PB = 262              # per-pair block cols: vp(102) + em(64) + qs(96)
PROJ_G = 4            # pairs per batched proj matmul


def _rel_position_index():
    ws = (8, 8)
    coords = np.stack(np.meshgrid(np.arange(ws[0]), np.arange(ws[1]), indexing="ij"))
    cf = coords.reshape(2, -1)
    rel = cf[:, :, None] - cf[:, None, :]
    rel = np.moveaxis(rel, 0, -1).astype(np.int64)
    rel[..., 0] += ws[0] - 1
    rel[..., 0] *= 2 * ws[1] - 1
    rel[..., 1] += ws[1] - 1
    return rel.sum(-1).reshape(-1)


def build_nc(nw_core: int, reps: int = 1, cp: int = CP, dyn: bool = True, stage: int = 3):
    tok = nw_core * N
    pairs = tok // PAIR_T
    cp = min(cp, pairs)
    n_chunk = pairs // cp
    assert pairs % cp == 0
    T = cp * PAIR_T                      # tokens per chunk
    X = 5 * T + PB * cp                  # slab cols (bf16)
    proj_g = min(PROJ_G, cp)
    assert cp % proj_g == 0
    GRP = min(4, n_chunk)
    assert n_chunk % GRP == 0
    n_iter = n_chunk // GRP

    nc = bacc.Bacc("TRN2", target_bir_lowering=False, debug=False)

    slab_d = nc.dram_tensor("slab", [(n_iter + 1) * 128, GRP * X], BF16,
                        kind="ExternalInput")
    biasT_d = nc.dram_tensor("biasT", [128, NH * N], BF16, kind="ExternalInput")
    wt_d = nc.dram_tensor("wt", [E, E], BF16, kind="ExternalInput")
    identb_d = nc.dram_tensor("identb", [128, 128], BF16, kind="ExternalInput")
    coff_d = nc.dram_tensor("coff", [E, 1], F32, kind="ExternalInput")
    yT_d = nc.dram_tensor("yT", [E, tok], BF16, kind="ExternalOutput")
    dbg_d = nc.dram_tensor("dbg", [128, 512], F32, kind="ExternalOutput") \
        if stage < 3 else None

    Exp = mybir.ActivationFunctionType.Exp
    Ln = mybir.ActivationFunctionType.Ln
    Ident = mybir.ActivationFunctionType.Identity

    with tile.TileContext(nc) as tc, ExitStack() as ctx:
        consts = ctx.enter_context(tc.tile_pool(name="consts", bufs=1))
        slabp = ctx.enter_context(tc.tile_pool(name="slabp", bufs=1))
        ytp = ctx.enter_context(tc.tile_pool(name="ytp", bufs=3))
        work = ctx.enter_context(tc.tile_pool(name="work", bufs=8))
        ps_s = ctx.enter_context(tc.tile_pool(name="ps_s", bufs=2, space="PSUM"))
        ps_a = ctx.enter_context(tc.tile_pool(name="ps_a", bufs=2, space="PSUM"))
        ps_t = ctx.enter_context(tc.tile_pool(name="ps_t", bufs=2, space="PSUM"))
        ps_z = ctx.enter_context(tc.tile_pool(name="ps_z", bufs=2, space="PSUM"))

        biasT = consts.tile([128, NH * N], BF16, tag="biasT")
        nc.sync.dma_start(out=biasT, in_=biasT_d[:, :])
        wt = consts.tile([E, E], BF16, tag="wt")
        nc.sync.dma_start(out=wt, in_=wt_d[:, :])
        identb = consts.tile([128, 128], BF16, tag="identb")
        nc.sync.dma_start(out=identb, in_=identb_d[:, :])
        coff = consts.tile([E, 1], F32, tag="coff")
        nc.sync.dma_start(out=coff, in_=coff_d[:, :])
        eps_t = consts.tile([128, 1], F32, tag="eps")
        nc.vector.memset(eps_t, EPS)

        def chunk_body(slab, yt, yoff, uniq):
            kT = slab[0:E, 0:T]
            qb = [slab[0:E, (1 + j) * T:(2 + j) * T] for j in range(4)]

            for g in range(cp // proj_g):
                xnT = work.tile([E, proj_g * PAIR_T], BF16, tag="xnT",
                                name=f"xnT_{uniq}_{g}")
                mvg = work.tile([PAIR_T, proj_g, 2], F32, tag="mvg",
                                name=f"mvg_{uniq}_{g}")
                bg = 5 * T + PB * (g * proj_g)
                blk4 = slab[:, bg:bg + proj_g * PB].rearrange(
                    "p (b r) -> p b r", b=proj_g)
                av4 = ps_a.tile([PAIR_T, proj_g, NH * 17], F32, tag="av",
                                name=f"av_{uniq}_{g}")
                for pg in range(proj_g):
                    p = g * proj_g + pg
                    t0 = PAIR_T * p
                    b0 = 5 * T + PB * p
                    vp = slab[:, b0:b0 + 102]
                    em = slab[:, b0 + 102:b0 + 166]

                    # ---- scores: one PSUM bank; partial-array QK weights use
                    # only bases {0, 64} (heads 0-3 as K=64 with per-head
                    # zero-padded q, heads 4-5 as K=32 at base 64) — mixing a
                    # base-32 load with others in one bank crashes the HW
                    sT = ps_s.tile([PAIR_T, NH * N], F32, tag="sT",
                                   name=f"sT_{uniq}_{p}")
                    nc.tensor.matmul(out=sT[:, :], lhsT=identb[:, :],
                                     rhs=biasT[:, :],
                                     start=True, stop=False,
                                     skip_group_check=True)
                    em_b = em.unsqueeze(1).broadcast_to([PAIR_T, NH, N])
                    nc.tensor.matmul(out=sT[:].rearrange("p (h i) -> p h i", h=NH),
                                     lhsT=identb[:, :], rhs=em_b,
                                     start=False, stop=False,
                                     skip_group_check=True)
                    for s in range(2):
                        ts0 = t0 + 64 * s
                        for h in range(NH):
                            last = (s == 1 and h == NH - 1)
                            if h < 4:
                                lhs = kT[0:64, ts0:ts0 + 64]
                                rhs = qb[h][0:64, ts0:ts0 + 64]
                            else:
                                lhs = kT[64:96, ts0:ts0 + 64]
                                rhs = qb[h - 4][64:96, ts0:ts0 + 64]
                            nc.tensor.matmul(
                                out=sT[64 * s:64 * s + 64, N * h:N * h + N],
                                lhsT=lhs, rhs=rhs,
                                start=False, stop=last,
                                skip_group_check=True)

                    # ---- P~ = exp(S+b+m), one drain -> bf16
                    e_t = work.tile([PAIR_T, NH * N], BF16, tag="e",
                                    name=f"e_{uniq}_{p}")
                    nc.scalar.activation(out=e_t[:, :], in_=sT[:, :], func=Exp)

                    if stage == 1:
                        dbge = work.tile([128, NH * N], F32, tag="dbge",
                                         name=f"dbge_{uniq}_{p}")
                        nc.vector.tensor_copy(out=dbge[:, :], in_=e_t[:, :])
                        nc.sync.dma_start(out=dbg_d[:, 0:NH * N], in_=dbge)
                        continue

                    # ---- AV with ones-augmented V: out + denominators
                    for s in range(2):
                        for h in range(NH):
                            nc.tensor.matmul(
                                out=av4[64 * s:64 * s + 64, pg, 17 * h:17 * h + 17],
                                lhsT=e_t[64 * s:64 * s + 64, N * h:N * h + N],
                                rhs=vp[64 * s:64 * s + 64, 17 * h:17 * h + 17])

                if stage == 1:
                    continue
                # ---- batched post-AV for the whole group (4 pairs at once)
                av_v = av4[:].rearrange("p g (h c) -> p g h c", h=NH)
                rec = work.tile([PAIR_T, proj_g, NH], F32, tag="rec",
                                name=f"rec_{uniq}_{g}")
                nc.vector.reciprocal(out=rec[:, :, :], in_=av_v[:, :, :, 16])
                x4 = work.tile([PAIR_T, proj_g, E], F32, tag="x",
                               name=f"x_{uniq}_{g}")
                x_v = x4[:].rearrange("p g (h c) -> p g h c", h=NH)
                rec_b = rec[:].unsqueeze(3).broadcast_to([PAIR_T, proj_g, NH, CH])
                nc.vector.tensor_tensor(out=x_v, in0=av_v[:, :, :, 0:16], in1=rec_b,
                                        op=mybir.AluOpType.mult)
                qs4 = blk4[:, :, 166:262]
                nc.vector.tensor_tensor(out=x4[:, :, :], in0=x4[:, :, :], in1=qs4,
                                        op=mybir.AluOpType.add)

                if stage == 2:
                    nc.sync.dma_start(out=dbg_d[:, 0:proj_g * E],
                                      in_=x4.reshape_hint if False else x4[:, :, :])
                    continue

                # ---- LayerNorm stats (walrus: bn_stats out must be 6/partition)
                for pg in range(proj_g):
                    stats = work.tile([PAIR_T, 6], F32, tag="stats",
                                      name=f"st_{uniq}_{g}_{pg}")
                    nc.vector.bn_stats(out=stats[:, :], in_=x4[:, pg, :])
                    nc.vector.bn_aggr(out=mvg[:, pg, :], in_=stats[:, :])

                # ---- rstd = rsqrt(var+eps): bit-trick seed + 1 Newton step,
                #      all int/mult DVE ALU ops (keeps ACT on one table set)
                tg = work.tile([PAIR_T, proj_g], F32, tag="tg",
                               name=f"tg_{uniq}_{g}")
                nc.vector.tensor_scalar(out=tg[:, :], in0=mvg[:, :, 1],
                                        scalar1=EPS, scalar2=None,
                                        op0=mybir.AluOpType.add)
                y0 = work.tile([PAIR_T, proj_g], F32, tag="y0",
                               name=f"y0_{uniq}_{g}")
                nc.vector.tensor_scalar(out=y0[:].bitcast(mybir.dt.int32),
                                        in0=tg[:].bitcast(mybir.dt.int32),
                                        scalar1=1, scalar2=-1,
                                        op0=mybir.AluOpType.logical_shift_right,
                                        op1=mybir.AluOpType.bitwise_xor)
                nc.vector.tensor_scalar(out=y0[:].bitcast(mybir.dt.int32),
                                        in0=y0[:].bitcast(mybir.dt.int32),
                                        scalar1=0x5F3759E0, scalar2=None,
                                        op0=mybir.AluOpType.add)
                d_t = work.tile([PAIR_T, proj_g], F32, tag="dt",
                                name=f"dt_{uniq}_{g}")
                nc.vector.tensor_tensor(out=d_t[:, :], in0=y0[:, :], in1=y0[:, :],
                                        op=mybir.AluOpType.mult)
                nc.vector.tensor_tensor(out=d_t[:, :], in0=d_t[:, :], in1=tg[:, :],
                                        op=mybir.AluOpType.mult)
                nc.vector.tensor_scalar(out=d_t[:, :], in0=d_t[:, :],
                                        scalar1=-0.5, scalar2=1.5,
                                        op0=mybir.AluOpType.mult,
                                        op1=mybir.AluOpType.add)
                rstd = work.tile([PAIR_T, proj_g], F32, tag="rstd",
                                 name=f"rstd_{uniq}_{g}")
                nc.vector.tensor_tensor(out=rstd[:, :], in0=y0[:, :], in1=d_t[:, :],
                                        op=mybir.AluOpType.mult)

                xnT_p = ps_t.tile([E, proj_g * PAIR_T], BF16, tag="tr",
                                  name=f"tr_{uniq}_{g}")
                for pg in range(proj_g):
                    p = g * proj_g + pg
                    xn = work.tile([PAIR_T, E], BF16, tag="xn", name=f"xn_{uniq}_{p}")
                    nc.vector.tensor_scalar(out=xn[:, :], in0=x4[:, pg, :],
                                            scalar1=mvg[:, pg, 0:1],
                                            scalar2=rstd[:, pg:pg + 1],
                                            op0=mybir.AluOpType.subtract,
                                            op1=mybir.AluOpType.mult)
                    # ---- transpose for proj (4 pairs share one PSUM tile)
                    nc.tensor.transpose(
                        out=xnT_p[:, PAIR_T * pg:PAIR_T * (pg + 1)], in_=xn[:, :],
                        identity=identb[:, :])
                nc.vector.tensor_copy(out=xnT[:, :], in_=xnT_p[:, :])

                # ---- proj over 4 pairs at once (N=512)
                zT = ps_z.tile([E, proj_g * PAIR_T], F32, tag="zT",
                               name=f"zT_{uniq}_{g}")
                nc.tensor.matmul(out=zT[:, :], lhsT=wt[:, :], rhs=xnT[:, :])
                c0 = yoff + proj_g * PAIR_T * g
                nc.scalar.activation(out=yt[:, c0:c0 + proj_g * PAIR_T],
                                     in_=zT[:, :], func=Ident, bias=coff[:, :])

        def group_body(ci, uniq, slab0):
            # GRP chunks per loop iteration.  Chunk 0's slab was prefetched
            # by the previous iteration (prologue for iter 0); chunks 1..GRP-1
            # DMA here and overlap chunk 0's compute.  At body end we prefetch
            # the NEXT iteration's chunk 0 into the same tile (slab_d has one
            # zero-padded tail block so ci+1 never reads out of bounds).
            slabs = [slab0]
            for j in range(1, GRP):
                sl = slabp.tile([128, X], BF16, tag=f"slab{j}",
                                name=f"slab_{uniq}_{j}")
                if dyn:
                    nc.sync.dma_start(out=sl,
                                      in_=slab_d[_ts(ci, 128), j * X:(j + 1) * X])
                else:
                    nc.sync.dma_start(
                        out=sl, in_=slab_d[ci * 128:(ci + 1) * 128,
                                           j * X:(j + 1) * X])
                slabs.append(sl)
            yt = ytp.tile([E, GRP * T], BF16, tag="yt", name=f"yt_{uniq}") \
                if stage >= 3 else None
            for j in range(GRP):
                chunk_body(slabs[j], yt, j * T, f"{uniq}_{j}")
            if dyn:
                nc.sync.dma_start(out=slab0,
                                  in_=slab_d[_ts(ci + 1, 128), 0:X])
            else:
                nc.sync.dma_start(
                    out=slab0, in_=slab_d[(ci + 1) * 128:(ci + 2) * 128, 0:X])
            if stage >= 3:
                if dyn:
                    nc.sync.dma_start(out=yT_d[:, _ts(ci, GRP * T)], in_=yt)
                else:
                    nc.sync.dma_start(
                        out=yT_d[:, ci * GRP * T:(ci + 1) * GRP * T], in_=yt)

        if dyn:
            hints = tuple(mybir.ALL_ENGINES)
            with tc.For_i(0, reps) as _rep:
                slab0 = slabp.tile([128, X], BF16, tag="slab0", name="slab0_d")
                nc.sync.dma_start(out=slab0, in_=slab_d[0:128, 0:X])
                with tc.For_i(0, n_iter, hint_engines=hints,
              staggered_reset=True) as ci:
                    group_body(ci, "d", slab0)
        else:
            for rep in range(reps):
                slab0 = slabp.tile([128, X], BF16, tag="slab0",
                                   name=f"slab0_{rep}")
                nc.sync.dma_start(out=slab0, in_=slab_d[0:128, 0:X])
                for ci in range(n_iter):
                    group_body(ci, f"{rep}_{ci}", slab0)

    nc.compile()
    return nc


def prepare_inputs(query, key, value, mask, bias_table, norm_gamma, norm_beta,
                   proj_w, proj_b, cp: int = CP):
    """Host-side data prep. Returns arrays shardable per-core along chunk axis."""
    nw = query.shape[0]
    tok = nw * N
    q2 = (query.astype(np.float32) * SCALE).reshape(tok, E)
    qT = q2.T                                          # [E, tok] f32 view
    kT = np.ascontiguousarray(
        key.astype(np.float32).reshape(tok, E).T).astype(BF)
    qb = np.zeros((4, E, tok), BF)
    for h in range(4):
        qb[h, CH * h:CH * h + CH] = qT[CH * h:CH * h + CH].astype(BF)
    qb[0, 64:80] = qT[64:80].astype(BF)     # head 4 rides block 0 rows 64-79
    qb[1, 80:96] = qT[80:96].astype(BF)     # head 5 rides block 1 rows 80-95

    vp = np.empty((tok, NH * 17), BF)
    v2 = value.reshape(tok, E)
    for h in range(NH):
        vp[:, 17 * h:17 * h + 16] = v2[:, 16 * h:16 * h + 16].astype(BF)
        vp[:, 17 * h + 16] = 1.0

    em = np.ascontiguousarray(
        mask.astype(np.float32).transpose(0, 2, 1)).reshape(tok, N).astype(BF)
    qs = q2.astype(BF)

    pairs = tok // PAIR_T
    cp = min(cp, pairs)
    n_chunk_total = pairs // cp
    T = cp * PAIR_T
    X = 5 * T + PB * cp

    slab = np.zeros((n_chunk_total, 128, X), BF)
    for ci in range(n_chunk_total):
        a = ci * T
        slab[ci, 0:E, 0:T] = kT[:, a:a + T]
        for j in range(4):
            slab[ci, 0:E, (1 + j) * T:(2 + j) * T] = qb[j, :, a:a + T]
        for p in range(cp):
            b0 = 5 * T + PB * p
            r = a + p * PAIR_T
            slab[ci, :, b0:b0 + 102] = vp[r:r + PAIR_T]
            slab[ci, :, b0 + 102:b0 + 166] = em[r:r + PAIR_T]
            slab[ci, :, b0 + 166:b0 + 262] = qs[r:r + PAIR_T]

    rel = _rel_position_index()
    bias = bias_table[rel].reshape(N, N, NH)                 # [i, j, h]
    bjhi = np.ascontiguousarray(bias.transpose(1, 2, 0)).reshape(N, NH * N)
    biasT = np.vstack([bjhi, bjhi]).astype(BF)               # [128, 384]

    weff = (proj_w * norm_gamma[None, :]).astype(np.float32)
    wt = np.ascontiguousarray(weff.T).astype(BF)             # [c, o]
    coff = (norm_beta @ proj_w.T + proj_b).astype(np.float32).reshape(E, 1)

    return {
        "slab": slab, "biasT": biasT, "wt": wt, "coff": coff,
        "identb": np.eye(128, dtype=BF),
    }


def core_in_maps(full, n_cores=N_CORES):
    n_chunk_total = full["slab"].shape[0]
    chunks_c = n_chunk_total // n_cores
    grp = min(4, chunks_c)
    maps = []
    for c in range(n_cores):
        sl = full["slab"][c * chunks_c:(c + 1) * chunks_c]
        X = sl.shape[2]
        sl = np.ascontiguousarray(
            sl.reshape(chunks_c // grp, grp, 128, X).transpose(0, 2, 1, 3))
        sl = sl.reshape(chunks_c // grp * 128, grp * X)
        sl = np.concatenate([sl, np.zeros((128, grp * X), BF)], axis=0)
        maps.append({
            "slab": sl,
            "biasT": full["biasT"], "wt": full["wt"],
            "coff": full["coff"], "identb": full["identb"],
        })
    return maps


_NC_CACHE = {}


def kernel(**inputs) -> np.ndarray:
    nw = inputs["query"].shape[0]
    assert nw % N_CORES == 0
    nw_c = nw // N_CORES

    full = prepare_inputs(**inputs)
    in_maps = core_in_maps(full)

    if nw_c not in _NC_CACHE:
        _NC_CACHE[nw_c] = build_nc(nw_c)
    nc = _NC_CACHE[nw_c]

    res = run_bass_kernel_spmd(nc, in_maps, core_ids=list(range(N_CORES)))
    yT = np.concatenate(
        [res.results[c]["yT"].astype(np.float32) for c in range(N_CORES)], axis=1)
    return np.ascontiguousarray(yT.T).reshape(nw, 8, 8, E).astype(np.float32)


if __name__ == "__main__":
    rng = np.random.default_rng(0)
    inputs = {
        "query": rng.standard_normal((NW, N, E), dtype=np.float32),
        "key": rng.standard_normal((NW, N, E), dtype=np.float32),
        "value": rng.standard_normal((NW, N, E), dtype=np.float32),
        "mask": rng.standard_normal((NW, N, N), dtype=np.float32),
        "bias_table": (rng.standard_normal((225, NH)) * 0.02).astype(np.float32),
        "norm_gamma": np.ones(E, np.float32),
        "norm_beta": np.zeros(E, np.float32),
        "proj_w": (rng.standard_normal((E, E)) * 0.02).astype(np.float32),
        "proj_b": np.zeros(E, np.float32),
    }
    print(kernel(**inputs).shape)
